# revision 52
# baseline (speedup 1.0000x reference)
"""Trainium2 Bass kernel for nn_Attention_17738214932808.

Computation (per batch b):
    mids   = q @ W.T                               [B, D]
    scores = tanh(k . mids + bias)                 [B, T]
    attn   = softmax-with-mask:  e = exp(scores - max) * m ; attn = e / sum(e)
Since tanh is bounded in (-1, 1), the max-subtraction is a mathematical no-op
for the final ratio (exp(s-c)/sum m exp(s-c) is invariant in c), so we compute
e = exp(scores) * m directly; fp32-rounding-level difference only.

Sharding: data-parallel over batch, 8 batches per NeuronCore x 8 cores.

Layout trick: each SBUF partition loads a CONTIGUOUS 16KB run of k (16 t-rows),
so k's DMA runs at ~HBM peak. The resulting score-column permutation is fixed
up by pre-permuting m and inverse-permuting the output on the host (pure input
marshalling; all FLOPs happen on-device).

Per-core engine split for the hot dot-product loop (16 subtiles per k-tile):
  - DVE: one big tensor_tensor multiply for 8 subtiles (products buffer),
    then 8 fused multiply+reduce (scalar_tensor_tensor w/ accum_out)
  - ACT: reduces the 8 products via activation(Copy, accum_out=), pipelined
    one step behind DVE's multiply
  - mids are precomputed ONCE, broadcast to all 128 partitions via one-hot
    selector matmuls on the otherwise-idle PE, and kept in SBUF (PSUM-source
    DVE ops pay +62 cycles)
  - the per-pair softmax epilogue is software-pipelined: its cross-engine
    tail is deferred into the next pair's first tile so ACT's in-order queue
    never parks on a dependency chain
This balances DVE (~84us), ACT (~84us) and k's DMA (~84us across 16 queues,
~410GB/s, the memory-roofline floor).  Measured best: ~110-111us/core HW exec
on 8 cores (baseline: ~132us); engine clocks on this part wander 0.8-1.0x
run-to-run, degraded samples measure ~122-125us.
"""

import os

import numpy as np

import concourse.bass as bass
import concourse.tile as tile
from concourse import bacc, mybir
from concourse.bass_utils import run_bass_kernel_spmd
from concourse.masks import make_identity

F32 = mybir.dt.float32
AF = mybir.ActivationFunctionType
ALU = mybir.AluOpType

B, T, D = 64, 4096, 256
NCORES = 8
BL = B // NCORES          # batches per core = 8
H = 2                     # halves of T per batch (macro tiles)
TT = 16                   # t-subtiles per macro  (T = H * 128 * TT)
P = 128

# Engine split per k-tile (16 subtiles): DVE multiplies N_DVEMUL subtiles with
# one big tensor_tensor (ACT then reduces each via activation+accum); the rest
# run fused multiply+accum (scalar_tensor_tensor) on DVE.  GpSimd streaming ops
# are avoided entirely: its SBUF port is shared with DVE and heavy GpSimd
# traffic stretches DVE ops ~6x (measured).  N_DVEMUL=8 balances DVE
# (~334ns/STT + ~2290ns big mult = ~79us) against ACT (~596ns/reduce incl
# READ_ACCUMULATOR = ~76us).
DVEMUL_PATTERN = (8, 8)   # per-tile ACT-reduce count: balances DVE ~82us vs ACT ~81us

LAST_RESULTS = None       # BassKernelResults of the most recent run (for test.py)


def _broadcast_row(ap, nparts):
    """[1, N] AP -> [nparts, N] AP with partition step 0."""
    try:
        return ap.to_broadcast([nparts] + list(ap.shape[1:]))
    except Exception:
        return bass.AP(
            tensor=ap.tensor,
            offset=ap.offset,
            ap=[[0, nparts]] + [list(d) for d in ap.ap[1:]],
        )


def _build_kernel(ctx, tc, outs, ins):
    nc = tc.nc
    q, k, mp, W, bias = ins["q"], ins["k"], ins["mp"], ins["W"], ins["bias"]
    out = outs["out"]

    consts = ctx.enter_context(tc.tile_pool(name="consts", bufs=1))
    setup = ctx.enter_context(tc.tile_pool(name="setup", bufs=1))
    kpool = ctx.enter_context(tc.tile_pool(name="kpool", bufs=8))
    scpool = ctx.enter_context(tc.tile_pool(name="scores", bufs=4))
    scratch = ctx.enter_context(tc.tile_pool(name="scratch", bufs=6))
    ascratch = ctx.enter_context(tc.tile_pool(name="ascratch", bufs=6))
    epool = ctx.enter_context(tc.tile_pool(name="epil", bufs=3))
    ps_misc = ctx.enter_context(tc.tile_pool(name="ps_misc", bufs=3, space="PSUM"))
    ps_e = ctx.enter_context(tc.tile_pool(name="ps_e", bufs=1, space="PSUM"))

    # ---------------- Phase 0: constants + mids = q @ W.T ----------------
    # Sync-ring (HWDGE) transfers run FIFO, so order matters: W and q first
    # (they gate the mids chain), then k tile 0 hoisted ahead of the rest of
    # the k stream.  Output stores ride GpSimd's SWDGE so they never insert
    # completion bubbles into the k-stream FIFO.
    w_sb = setup.tile([P, 2, D], F32, tag="w")
    nc.sync.dma_start(out=w_sb[:], in_=W.ap().rearrange("(dc p) e -> p dc e", p=P))
    q_sb = setup.tile([BL, D], F32, tag="q")
    nc.sync.dma_start(out=q_sb[:], in_=q.ap())
    # tile 0 arrives as two half DMAs, multiply-path half (subtiles 8:16)
    # first: the big multiply can start ~2.5us before the full tile lands
    kt0 = kpool.tile([P, TT, D], F32, tag="ktile")
    kt0_src = k.ap()[0, 0:2048, :].rearrange("(p tt) d -> p tt d", p=P)
    nc.sync.dma_start(out=kt0[:, 8:TT, :], in_=kt0_src[:, 8:TT, :])
    nc.sync.dma_start(out=kt0[:, 0:8, :], in_=kt0_src[:, 0:8, :])

    ident = consts.tile([P, P], F32)
    make_identity(nc, ident)

    bias_col = consts.tile([P, 1], F32)
    nc.gpsimd.dma_start(out=bias_col[:], in_=_broadcast_row(bias.ap(), P))

    # block-diagonal ones [64, 2, 32] (m = 32g + j): blk64[p, g, j] = 1 iff
    # p//32 == g.  One matmul with this as lhsT turns the per-row masked sums
    # rs[64, 1] into per-batch totals broadcast back to all 64 rows.
    blk64 = consts.tile([64, 2, 32], F32)
    nc.gpsimd.memset(blk64[:], 1.0)
    nc.gpsimd.affine_select(   # keep where p - 32g >= 0
        out=blk64[:], in_=blk64[:], compare_op=ALU.is_ge, fill=0.0,
        base=0, pattern=[[-32, 2], [0, 32]], channel_multiplier=1,
    )
    nc.gpsimd.affine_select(   # keep where 31 - p + 32g >= 0  (p - 32g <= 31)
        out=blk64[:], in_=blk64[:], compare_op=ALU.is_ge, fill=0.0,
        base=31, pattern=[[32, 2], [0, 32]], channel_multiplier=-1,
    )

    # W^T chunks: wt[p=e_local, ec, dc, d_local]
    wt = setup.tile([P, 2, 2, P], F32, tag="wt")
    for dc in range(2):
        for ec in range(2):
            pst = ps_misc.tile([P, P], F32, tag="mix")
            nc.tensor.transpose(pst[:], w_sb[:, dc, ec * P:(ec + 1) * P], ident[:])
            nc.vector.tensor_copy(wt[:, ec, dc, :], pst[:])
    # q^T chunks: qt[p=e_local, ec, b]
    qt = setup.tile([P, 2, BL], F32, tag="qt")
    for ec in range(2):
        pst = ps_misc.tile([P, BL], F32, tag="mix")
        nc.tensor.transpose(pst[:], q_sb[:, ec * P:(ec + 1) * P], ident[0:BL, 0:BL])
        nc.vector.tensor_copy(qt[:, ec, :], pst[:])

    # mids rows [8, 256] directly: mids[b, d] = sum_e qT[e, b] * wT[e, d]
    mids = setup.tile([BL, D], F32, tag="mids")
    psm = ps_e.tile([BL, D], F32, tag="mids_ps")
    for ec in range(2):
        nc.tensor.matmul(
            psm[:], lhsT=qt[:, ec, :],
            rhs=wt[:, ec, :, :].rearrange("p dc d -> p (dc d)"),
            start=(ec == 0), stop=(ec == 1),
        )
    nc.vector.tensor_copy(mids[:], psm[:])

    # one-hot selector for the mids broadcast: selb[p, b, m] = 1 iff p == b
    selb = consts.tile([BL, BL, P], F32)
    nc.gpsimd.memset(selb[:], 0.0)
    nc.gpsimd.affine_select(   # iota = p - b; keep (!=0) -> stays 0, eq -> fill 1
        out=selb[:], in_=selb[:], compare_op=ALU.not_equal, fill=1.0,
        base=0, pattern=[[-1, BL], [0, P]], channel_multiplier=1,
    )

    # ---------------- Phase 0b: broadcast ALL mids to SBUF [P, BL, D] ----------------
    # Selector matmuls: mb[p, d] = sum_{p'<8} selb[p', b, p] * mids[p', d]
    # replicate each mids row across all 128 partitions via PSUM; ACT copies
    # the PSUM pairs into one SBUF tile so every main-loop operand read is
    # SBUF (PSUM-source DVE ops pay +62 cycles each).
    ps_mb = ctx.enter_context(tc.tile_pool(name="ps_mb", bufs=2, space="PSUM"))
    mbsb = setup.tile([P, BL, D], F32, tag="mbsb")
    mb_ps0 = None
    for g in range(BL // 2):
        mb_ps = ps_mb.tile([P, 2, D], F32)
        for b_local in range(2):
            b = g * 2 + b_local
            nc.tensor.matmul(
                mb_ps[:, b_local, :], lhsT=selb[:, b, :],
                rhs=mids[:], start=True, stop=True,
            )
        nc.scalar.copy(mbsb[:, g * 2:(g + 1) * 2, :], mb_ps[:])
        if g == 0:
            mb_ps0 = mb_ps

    # ---------------- Phase 1: main loop + epilogue per batch-pair ----------------
    # Per k-tile [128 t-rows, 16 subtiles, 256 d]:
    #   - DVE: one big multiply for the N_DVEMUL subtiles (products -> prodpool),
    #     then the fused multiply+accum (scalar_tensor_tensor) subtiles
    #   - ACT: N_DVEMUL per-subtile reduces of the products (runs one step
    #     behind DVE's multiply, overlapping with DVE's STTs and the next tile)
    # The per-pair epilogue is software-pipelined: tanh/exp/transpose issue
    # right after the pair's last tile, but the cross-engine tail (masked sum,
    # reciprocal, final scale, store) is deferred into the NEXT pair's first
    # tile so ACT's in-order queue is never parked on a dependency chain.
    prodpool = ctx.enter_context(tc.tile_pool(name="prod", bufs=5))

    def emit_tile(b, b_local, h, scores, kt=None, mb=None):
        if mb is None:
            mb = mbsb[:, b, :]
        if kt is None:
            kt = kpool.tile([P, TT, D], F32, tag="ktile")
            nc.sync.dma_start(
                out=kt[:],
                in_=k.ap()[b, h * 2048:(h + 1) * 2048, :].rearrange(
                    "(p tt) d -> p tt d", p=P
                ),
            )
        c0 = b_local * 32 + h * 16
        n_dvemul = DVEMUL_PATTERN[h]
        n_stt = TT - n_dvemul
        # DVE: the multiply for subtiles [n_stt, 16), split in two halves so
        # ACT's reduces can start after the first half
        prod = prodpool.tile([P, max(DVEMUL_PATTERN), D], F32, tag="prod")
        nh = n_dvemul // 2
        for lo, hi in ((0, nh), (nh, n_dvemul)):
            nc.vector.tensor_tensor(
                out=prod[:, lo:hi, :],
                in0=kt[:, n_stt + lo:n_stt + hi, :],
                in1=mb.unsqueeze(1).broadcast_to([P, hi - lo, D]),
                op=ALU.mult,
            )
            # ACT: reduce every multiplied subtile to its scores column
            for i in range(lo, hi):
                asc = ascratch.tile([P, D], F32, tag="actred")
                nc.scalar.activation(
                    out=asc[:], in_=prod[:, i, :], func=AF.Copy,
                    accum_out=scores[:, c0 + n_stt + i:c0 + n_stt + i + 1],
                )
        # DVE: fused multiply+accum for the remaining subtiles
        for tt in range(n_stt):
            sc = scratch.tile([P, D], F32, tag="ttr")
            nc.vector.scalar_tensor_tensor(
                out=sc[:], in0=kt[:, tt, :], scalar=0.0, in1=mb,
                op0=ALU.bypass, op1=ALU.mult,
                accum_out=scores[:, c0 + tt:c0 + tt + 1],
            )

    def epilogue_tail(g, pse, mt):
        ee = epool.tile([64, P], F32, tag="ee")
        rs = epool.tile([64, 1], F32, tag="rs")
        nc.vector.scalar_tensor_tensor(
            out=ee[:], in0=pse[:], scalar=0.0, in1=mt[:],
            op0=ALU.bypass, op1=ALU.mult, accum_out=rs[:],
        )
        pss = ps_misc.tile([64, 1], F32, tag="mix")
        nc.tensor.matmul(
            pss[:], lhsT=blk64.rearrange("p g j -> p (g j)"), rhs=rs[:],
            start=True, stop=True,
        )
        rcol = epool.tile([64, 1], F32, tag="rcol")
        nc.vector.reciprocal(rcol[:], pss[:])
        attn = epool.tile([64, P], F32, tag="attn")
        nc.scalar.activation(out=attn[:], in_=ee[:], func=AF.Copy, scale=rcol[:])
        # the last pair's store rides the sync HWDGE ring (empty once the k
        # stream is done): ~1.5us faster completion than GpSimd's SWDGE,
        # which still carries the overlapped mid-run stores
        eng = nc.sync if g == BL // 2 - 1 else nc.gpsimd
        eng.dma_start(
            out=out.ap()[g * 2:(g + 1) * 2].rearrange("b c p -> (b c) p"),
            in_=attn[:],
        )

    pending_tail = None
    for g in range(BL // 2):                 # 4 pairs
        scores = scpool.tile([P, 64], F32)   # col = b_local*32 + h*16 + tt
        # prefetch this pair's mask tile well ahead of its use
        mt = epool.tile([64, P], F32, tag="mt")
        nc.gpsimd.dma_start(
            out=mt[:],
            in_=mp.ap()[g * 2:(g + 1) * 2].rearrange("b c p -> (b c) p"),
        )
        for b_local in range(2):
            b = g * 2 + b_local
            for h in range(2):
                first = (g, b_local, h) == (0, 0, 0)
                # the very first tile reads mb straight from PSUM (+62c per
                # DVE op) rather than waiting for the ACT copy into mbsb
                emit_tile(b, b_local, h, scores,
                          kt=kt0 if first else None,
                          mb=mb_ps0[:, 0, :] if first else None)
                if pending_tail is not None:
                    pending_tail()
                    pending_tail = None

        # ---- epilogue head for this pair: tanh/exp/transpose ----
        th = epool.tile([P, 64], F32, tag="th")
        nc.scalar.activation(out=th[:], in_=scores[:], func=AF.Tanh,
                             bias=bias_col[:], scale=1.0)
        ex = epool.tile([P, 64], F32, tag="ex")
        nc.scalar.activation(out=ex[:], in_=th[:], func=AF.Exp)
        pse = ps_e.tile([64, P], F32)
        nc.tensor.transpose(pse[:], ex[:], ident[:])
        pending_tail = (lambda g=g, pse=pse, mt=mt: epilogue_tail(g, pse, mt))

    pending_tail()


def _install_ntff_hook_shim():
    """Provide antenv.axon_hooks via ctypes into libaxon_pjrt.so (the agent
    image's antenv stub lacks it), enabling NTFF capture under trace=True."""
    import sys
    import types
    import ctypes
    import contextlib

    if "antenv.axon_hooks" in sys.modules:
        return
    so = "/opt/axon/libaxon_pjrt.so"
    if not os.path.exists(so):
        return
    lib = ctypes.CDLL(so)
    if not hasattr(lib, "axon_start_nrt_profile"):
        return
    lib.axon_start_nrt_profile.argtypes = [
        ctypes.POINTER(ctypes.c_int64), ctypes.c_size_t,
    ]
    lib.axon_start_nrt_profile.restype = ctypes.c_int64
    lib.axon_stop_nrt_profile.argtypes = [ctypes.c_char_p]
    lib.axon_stop_nrt_profile.restype = ctypes.c_int64

    @contextlib.contextmanager
    def _hook(output_dir, device_ids):
        import jax

        jax.devices()
        if device_ids:
            ids = (ctypes.c_int64 * len(device_ids))(*device_ids)
            rc = lib.axon_start_nrt_profile(ids, len(device_ids))
        else:
            rc = lib.axon_start_nrt_profile(None, 0)
        if rc != 0:
            raise RuntimeError(f"axon_start_nrt_profile rc={rc}")
        try:
            yield
        finally:
            n = lib.axon_stop_nrt_profile(str(output_dir).encode())
            print(f"profile: {n} file(s) written to {output_dir}", file=sys.stderr)

    mod = types.ModuleType("antenv.axon_hooks")
    mod.get_axon_ntff_profile_hook = lambda: _hook
    mod.set_axon_ntff_profile_hook = lambda h: None
    import antenv

    sys.modules["antenv.axon_hooks"] = mod
    antenv.axon_hooks = mod


_CACHE = {}


def _get_nc():
    if "nc" not in _CACHE:
        from contextlib import ExitStack

        nc = bacc.Bacc("TRN2", debug=False)
        ins = {
            "q": nc.dram_tensor("q", [BL, D], F32, kind="ExternalInput"),
            "k": nc.dram_tensor("k", [BL, T, D], F32, kind="ExternalInput"),
            "mp": nc.dram_tensor("mp", [BL, 32, P], F32, kind="ExternalInput"),
            "W": nc.dram_tensor("W", [D, D], F32, kind="ExternalInput"),
            "bias": nc.dram_tensor("bias", [1, 1], F32, kind="ExternalInput"),
        }
        outs = {"out": nc.dram_tensor("out", [BL, 32, P], F32, kind="ExternalOutput")}
        with tile.TileContext(nc) as tc:
            with ExitStack() as ctx:
                _build_kernel(ctx, tc, outs, ins)
        nc.compile()
        _CACHE["nc"] = nc
    return _CACHE["nc"]


def kernel(q, k, m, W, bias):
    global LAST_RESULTS
    q = np.ascontiguousarray(q, dtype=np.float32)
    k = np.ascontiguousarray(k, dtype=np.float32)
    m = np.ascontiguousarray(m, dtype=np.float32)
    W = np.ascontiguousarray(W, dtype=np.float32)
    bias = np.ascontiguousarray(bias, dtype=np.float32).reshape(1, 1)

    # host-side input marshalling: permute m to the kernel's score layout.
    # DVE/ACT macros: mp[b, h*16+tt, p] = m[b, h*2048 + p*16 + tt]
    # PE macros (even local batch, h=0): mp[b, tt, p] = m[b, tt*128 + p]
    mp = np.ascontiguousarray(
        m.reshape(B, H, P, TT).transpose(0, 1, 3, 2).reshape(B, H * TT, P)
    )
    trace = bool(int(os.environ.get("KERNEL_TRACE", "0")))
    if trace:
        _install_ntff_hook_shim()
    nc = _get_nc()
    in_maps = [
        {
            "q": q[i * BL:(i + 1) * BL],
            "k": k[i * BL:(i + 1) * BL],
            "mp": mp[i * BL:(i + 1) * BL],
            "W": W,
            "bias": bias,
        }
        for i in range(NCORES)
    ]
    res = run_bass_kernel_spmd(
        nc,
        in_maps,
        core_ids=list(range(NCORES)),
        trace=trace,
    )
    LAST_RESULTS = res

    full = np.concatenate([res.results[i]["out"] for i in range(NCORES)], axis=0)
    # inverse permutation back to natural [B, T]
    out = np.ascontiguousarray(
        full.reshape(B, H, TT, P).transpose(0, 1, 3, 2).reshape(B, T)
    )
    return out

